# revision 1
# baseline (speedup 1.0000x reference)
"""Trainium2 Bass kernel for batched multi-head attention (B=2, S=2048, E=1024, H=16).

Sharding: core r = 4*b + g handles batch b and head-group g (4 heads, 256 emb cols).
- QKV projections: tensor-parallel over head groups (each core computes its 256
  output cols from the full 1024-dim input, streamed in 128-row blocks).
- Attention: each core runs 4 heads over all 2048 queries (scores kept transposed
  [kt, qt]; softmax normalization deferred via a mask/ones column appended to V).
- Out-proj: token-parallel. Contexts are exchanged with a single 8-rank AllToAll
  (cross-batch slots zeroed via a host-fed 0/1 zmask so the program stays SPMD-
  uniform; the receiver sums slot pairs j and j+4). Each core then computes its
  512-token slice of the output.
All matmuls run in float32r (full-rate fp32, round-to-nearest-even at 11 mantissa
bits, ~1.5e-4 rel err). Inputs are pre-rounded to f32r bit patterns on the host,
so no on-chip rounding passes are needed.
"""

import sys

if '/opt/trn_rl_repo' not in sys.path:
    sys.path.insert(0, '/opt/trn_rl_repo')

import numpy as np

P = 128
B, S, E, H, DH = 2, 2048, 1024, 16, 64
NCORES = 8
G = 4                 # head groups == cores per batch
EG = E // G           # 256 emb cols per group
TS = S // G           # 512 tokens per core in out-proj
KB = S // P           # 16 key-token blocks
IB = E // P           # 8 contraction blocks of 128
QW = 512              # matmul moving free-dim chunk
SCALE = DH ** -0.5
ATT_DT = 'f32r'        # upstream dtype: 'f32r' | 'fp16' | 'bf16'

_cache = {}


def _round_f32r(x):
    """Round fp32 array to float32r bit patterns (RNE, 11 mantissa bits)."""
    i = np.ascontiguousarray(x, np.float32).view(np.uint32).astype(np.uint64)
    bias = ((i >> 12) & 1) + 0x7FF
    return (((i + bias) >> 12) << 12).astype(np.uint32).view(np.float32)


def _patch_ldw_opt():
    import concourse.bass_utils as bu
    if getattr(bu, '_ldw_patched', False):
        return
    orig = bu.run_command
    bu._ldw_patched = True


def _build():
    import concourse.bass as bass
    import concourse.mybir as mybir
    import concourse.tile as tile
    from concourse import bacc
    from contextlib import ExitStack

    f32 = mybir.dt.float32
    f32r = mybir.dt.float32r
    bf16 = {'f32r': mybir.dt.float32r, 'fp16': mybir.dt.float16,
            'bf16': mybir.dt.bfloat16}[ATT_DT]
    AF = mybir.ActivationFunctionType

    nc = bacc.Bacc("TRN2", target_bir_lowering=False, debug=False,
                   num_devices=NCORES)

    xqT = nc.dram_tensor("xqT", [E, S], bf16, kind="ExternalInput").ap()
    xkT = nc.dram_tensor("xkT", [E, S], bf16, kind="ExternalInput").ap()
    xvT = nc.dram_tensor("xvT", [E, S], bf16, kind="ExternalInput").ap()
    wqT = nc.dram_tensor("wqT", [E, EG], bf16, kind="ExternalInput").ap()
    wkT = nc.dram_tensor("wkT", [E, EG], bf16, kind="ExternalInput").ap()
    wvT = nc.dram_tensor("wvT", [E, EG], bf16, kind="ExternalInput").ap()
    woT = nc.dram_tensor("woT", [E, E], f32r, kind="ExternalInput").ap()
    bqv = nc.dram_tensor("bqv", [1, EG], bf16, kind="ExternalInput").ap()
    bkv = nc.dram_tensor("bkv", [1, EG], bf16, kind="ExternalInput").ap()
    bvv = nc.dram_tensor("bvv", [1, EG], bf16, kind="ExternalInput").ap()
    bov = nc.dram_tensor("bov", [1, E], f32r, kind="ExternalInput").ap()
    onesv = nc.dram_tensor("onesv", [1, QW], f32r, kind="ExternalInput").ap()
    onesb = nc.dram_tensor("onesb", [1, QW], bf16, kind="ExternalInput").ap()
    mask_pb = nc.dram_tensor("mask_pb", [P, KB], f32, kind="ExternalInput").ap()
    maskrep = nc.dram_tensor("maskrep", [P, KB * G], f32, kind="ExternalInput").ap()
    zmask = nc.dram_tensor("zmask", [P, NCORES], f32, kind="ExternalInput").ap()
    out = nc.dram_tensor("out", [TS, E], f32, kind="ExternalOutput").ap()

    a2a_ins = [nc.dram_tensor(f"a2a_in{h}", [NCORES, DH, TS], f32r).ap()
               for h in range(G)]
    a2a_outs = [nc.dram_tensor(f"a2a_out{h}", [NCORES, DH, TS], f32r).ap()
                for h in range(G)]

    with tile.TileContext(nc) as tc, ExitStack() as top:
        const = top.enter_context(tc.tile_pool(name="const", bufs=1))

        ones_r = const.tile([1, QW], f32r)
        nc.sync.dma_start(ones_r[:], onesv[:])
        ones_b = const.tile([1, QW], bf16)
        nc.sync.dma_start(ones_b[:], onesb[:])
        bq_r = const.tile([1, EG], bf16)
        nc.sync.dma_start(bq_r[:], bqv[:])
        bk_r = const.tile([1, EG], bf16)
        nc.sync.dma_start(bk_r[:], bkv[:])
        bv_r = const.tile([1, EG], bf16)
        nc.sync.dma_start(bv_r[:], bvv[:])
        bo_r = const.tile([1, E], f32r)
        nc.sync.dma_start(bo_r[:], bov[:])
        mask_t = const.tile([P, KB], f32)
        nc.sync.dma_start(mask_t[:], mask_pb[:])
        maskrep_t = const.tile([P, KB * G], f32)
        nc.sync.dma_start(maskrep_t[:], maskrep[:])
        zmask_t = const.tile([P, NCORES], f32)
        nc.sync.dma_start(zmask_t[:], zmask[:])

        # persistent projection outputs
        proj_sb = top.enter_context(tc.tile_pool(name="proj_sb", bufs=1))
        qpT = [proj_sb.tile([P, S], bf16, tag=f"qpT{m}", name=f"qpT{m}")
               for m in range(2)]
        kpT = [proj_sb.tile([P, S], bf16, tag=f"kpT{m}", name=f"kpT{m}")
               for m in range(2)]
        # vp tiles: per kt-block, [P, 4 heads x (64 vals + 1 mask col)]
        vp_sb = [proj_sb.tile([P, G * (DH + 1)], bf16, tag=f"vp{m}", name=f"vp{m}")
                 for m in range(KB)]

        # ---- projection weights ----
        with tc.tile_pool(name="wqkv", bufs=1) as wqkv:
            w_r = {}
            for name, wap in (("q", wqT), ("k", wkT), ("v", wvT)):
                wr = wqkv.tile([P, IB * EG], bf16, tag=f"w{name}r", name=f"w{name}r")
                for i in range(IB):
                    nc.sync.dma_start(wr[:, i * EG:(i + 1) * EG],
                                      wap[i * P:(i + 1) * P, :])
                w_r[name] = wr

            # ---- K and Q projections: out [e_sel, t] transposed ----
            with tc.tile_pool(name="vpT_p", bufs=1) as vpT_p:
              vpT = [vpT_p.tile([P, S], bf16, tag=f"vpT{m}", name=f"vpT{m}")
                     for m in range(2)]
              with tc.tile_pool(name="xst", bufs=3) as xst, \
                 tc.tile_pool(name="kqpsum", bufs=1, space="PSUM") as kqpsum:
                for name, xap, brow, dsts in (("v", xvT, bv_r, vpT),
                                              ("k", xkT, bk_r, kpT),
                                              ("q", xqT, bq_r, qpT)):
                    kqs = [kqpsum.tile([P, S], f32, tag=f"kqs{m}", name=f"kqs{m}")
                           for m in range(2)]
                    for i in range(IB):
                        xr = xst.tile([P, S], bf16)
                        nc.sync.dma_start(xr[:], xap[i * P:(i + 1) * P, :])
                        for m in range(2):
                            for c in range(S // QW):
                                nc.tensor.matmul(
                                    kqs[m][:, c * QW:(c + 1) * QW],
                                    w_r[name][:, i * EG + m * P:i * EG + (m + 1) * P],
                                    xr[:, c * QW:(c + 1) * QW],
                                    start=(i == 0), stop=False)
                    for m in range(2):
                        for c in range(S // QW):
                            nc.tensor.matmul(
                                kqs[m][:, c * QW:(c + 1) * QW],
                                brow[0:1, m * P:(m + 1) * P],
                                ones_b[:],
                                start=False, stop=True)
                    for m in range(2):
                        nc.vector.tensor_copy(dsts[m][:], kqs[m][:])

              # ---- transpose vpT [e,t] -> token-major vp tiles, 32x32 blocks
              with tc.tile_pool(name="vtok_p", bufs=1) as vtok_p:
                vp_tok = vtok_p.tile([P, KB * EG], bf16)
                SQ = 32
                for m2 in range(2):
                    for a in range(P // SQ):
                        for b_ in range(P // SQ):
                            dst = vp_tok[a * SQ:(a + 1) * SQ, :] \
                                .rearrange("p (kt e) -> p kt e", e=EG)[
                                    :, :, m2 * P + b_ * SQ:m2 * P + (b_ + 1) * SQ]
                            srcb = vpT[m2][b_ * SQ:(b_ + 1) * SQ, :] \
                                .rearrange("p (kt t) -> p kt t", t=P)[
                                    :, :, a * SQ:(a + 1) * SQ]
                            nc.vector.transpose(dst.bitcast(f32), srcb.bitcast(f32))
                for m in range(KB):
                    src3 = vp_tok[:, m * EG:(m + 1) * EG] \
                        .rearrange("p (h d) -> p h d", h=G)
                    dst3 = vp_sb[m].rearrange("p (h e) -> p h e", e=DH + 1)[:, :, 0:DH]
                    nc.vector.tensor_scalar_mul(dst3, src3, mask_t[:, m:m + 1])
                    nc.vector.tensor_copy(
                        vp_sb[m].rearrange("p (h e) -> p h e", e=DH + 1)[:, :, DH:DH + 1],
                        maskrep_t[:, m * G:(m + 1) * G].rearrange("p (h e) -> p h e", e=1))

        # ---- out-proj weights: loaded during attention ----
        wo_pool = top.enter_context(tc.tile_pool(name="wo", bufs=1))
        wo_r = wo_pool.tile([P, IB * E], f32r)
        for i in range(IB):
            nc.sync.dma_start(wo_r[:, i * E:(i + 1) * E], woT[i * P:(i + 1) * P, :])

        # ---- attention ----
        HW = 1024  # qt half width
        with tc.tile_pool(name="spsum", bufs=2, space="PSUM") as spsum, \
             tc.tile_pool(name="pvpsum", bufs=2, space="PSUM") as pvpsum, \
             tc.tile_pool(name="expp", bufs=3) as expp, \
             tc.tile_pool(name="normp", bufs=2) as normp, \
             tc.tile_pool(name="sendp", bufs=3) as sendp:
            for h in range(G):
                mt = h // 2          # which qpT/kpT tile
                po = (h % 2) * DH    # partition offset within it
                for half in range(2):
                    pv = pvpsum.tile([DH + 1, HW], f32)
                    for j in range(KB):
                        sp = spsum.tile([P, HW], f32)
                        for c in range(HW // QW):
                            q0 = half * HW + c * QW
                            nc.tensor.matmul(
                                sp[:, c * QW:(c + 1) * QW],
                                kpT[mt][po:po + DH, j * P:(j + 1) * P],
                                qpT[mt][po:po + DH, q0:q0 + QW],
                                start=True, stop=True)
                        es = expp.tile([P, HW], bf16)
                        nc.scalar.activation(es[:], sp[:], AF.Exp, scale=SCALE)
                        for c in range(HW // QW):
                            nc.tensor.matmul(
                                pv[:, c * QW:(c + 1) * QW],
                                vp_sb[j][:, h * (DH + 1):(h + 1) * (DH + 1)],
                                es[:, c * QW:(c + 1) * QW],
                                start=(j == 0), stop=(j == KB - 1))
                    # normalize: rec = 1/sumexp (row DH), broadcast, multiply
                    rec = normp.tile([1, HW], f32, tag="rec", name="rec")
                    nc.vector.reciprocal(rec[:], pv[DH:DH + 1, :])
                    recB = normp.tile([DH, HW], f32, tag="recB", name="recB")
                    nc.gpsimd.partition_broadcast(recB[:], rec[:])
                    ctxn = normp.tile([DH, HW], f32r, tag="ctxn", name="ctxn")
                    nc.vector.tensor_mul(ctxn[:], pv[0:DH, :], recB[:])
                    # send slots: j%4 in {2*half, 2*half+1}; zero cross-batch
                    for jj in range(2):
                        shard = half * 2 + jj
                        for slot in (shard, shard + 4):
                            st = sendp.tile([DH, TS], f32r)
                            nc.vector.tensor_scalar_mul(
                                st[:], ctxn[:, jj * TS:(jj + 1) * TS],
                                zmask_t[0:DH, slot:slot + 1])
                            nc.sync.dma_start(a2a_ins[h][slot], st[:])
                # fire this head's exchange as soon as both halves are sent
                if half == 1:
                    nc.gpsimd.collective_compute(
                        "AllToAll", mybir.AluOpType.bypass,
                        replica_groups=[list(range(NCORES))],
                        ins=[a2a_ins[h][:]], outs=[a2a_outs[h][:]])

        # ---- out-proj on my 512-token slice ----
        with tc.tile_pool(name="gap", bufs=1) as gap, \
             tc.tile_pool(name="gstage", bufs=4) as gstage, \
             tc.tile_pool(name="opsum", bufs=2, space="PSUM") as opsum, \
             tc.tile_pool(name="outsb", bufs=2) as outsb:
            ga = {}
            ib_order = []
            for m in range(EG // P):       # m-major: collective availability
                for gp in range(G):
                    ib = gp * 2 + m
                    gt = gap.tile([P, TS], f32r, tag=f"ga{ib}", name=f"ga{ib}")
                    for hh in (2 * m, 2 * m + 1):
                        t0 = gstage.tile([DH, TS], f32r, tag="g0", name="g0")
                        t1 = gstage.tile([DH, TS], f32r, tag="g1", name="g1")
                        nc.sync.dma_start(t0[:], a2a_outs[hh][gp])
                        nc.sync.dma_start(t1[:], a2a_outs[hh][gp + 4])
                        r0 = (hh % 2) * DH
                        nc.vector.tensor_add(gt[r0:r0 + DH, :], t0[:], t1[:])
                    ga[ib] = gt
                    ib_order.append(ib)
            for tm in range(TS // P):
                pot = opsum.tile([P, E], f32)
                for n, ib in enumerate(ib_order):
                    for oc in range(E // QW):
                        nc.tensor.matmul(
                            pot[:, oc * QW:(oc + 1) * QW],
                            ga[ib][:, tm * P:(tm + 1) * P],
                            wo_r[:, ib * E + oc * QW:ib * E + oc * QW + QW],
                            start=(n == 0), stop=False)
                for oc in range(E // QW):
                    nc.tensor.matmul(
                        pot[:, oc * QW:(oc + 1) * QW],
                        ones_r[0:1, 0:P],
                        bo_r[0:1, oc * QW:(oc + 1) * QW],
                        start=False, stop=True)
                ot = outsb.tile([P, E], f32)
                nc.scalar.activation(ot[:], pot[:], AF.Copy)
                nc.sync.dma_start(out[tm * P:(tm + 1) * P, :], ot[:])

    nc.compile()
    return nc


def _get_nc():
    if 'nc' not in _cache:
        _cache['nc'] = _build()
    return _cache['nc']


def kernel(q, k, v, mask, Wq, bq, Wk, bk, Wv, bv, Wo, bo):
    from concourse.bass_utils import run_bass_kernel_spmd

    _patch_ldw_opt()
    nc = _get_nc()
    q = np.asarray(q, np.float32)
    k = np.asarray(k, np.float32)
    v = np.asarray(v, np.float32)
    f32 = np.float32

    import ml_dtypes
    if ATT_DT == 'f32r':
        bft = None  # keep f32 bits, pre-rounded to f32r
        def _cvt(x):
            return _round_f32r(x)
    else:
        bft = np.float16 if ATT_DT == 'fp16' else ml_dtypes.bfloat16
        def _cvt(x):
            return np.ascontiguousarray(x, np.float32).astype(bft)
    qT = [_cvt(q[b].T) for b in range(B)]
    kT = [_cvt(k[b].T) for b in range(B)]
    vT = [_cvt(v[b].T) for b in range(B)]
    WqT = _cvt(np.asarray(Wq, f32).T)
    WkT = _cvt(np.asarray(Wk, f32).T)
    WvT = _cvt(np.asarray(Wv, f32).T)
    WoT = _round_f32r(np.asarray(Wo, f32).T)
    bq = _cvt(np.asarray(bq, f32))
    bk = _cvt(np.asarray(bk, f32))
    bv = _cvt(np.asarray(bv, f32))
    bo = _round_f32r(np.asarray(bo, f32))
    onesr = np.ones((1, QW), f32)
    onesbv = np.ones((1, QW), np.float32 if bft is None else bft)
    maskf = (np.asarray(mask) != 0).astype(f32)  # [B, S]

    in_maps = []
    for r in range(NCORES):
        b, g = r // G, r % G
        cols = slice(g * EG, (g + 1) * EG)
        m_pb = np.ascontiguousarray(maskf[b].reshape(KB, P).T)       # [128,16]
        m_rep = np.ascontiguousarray(np.repeat(m_pb, G, axis=1))     # [128,64]
        zm = np.zeros((P, NCORES), f32)
        zm[:, b * G:(b + 1) * G] = 1.0
        in_maps.append({
            "xqT": qT[b], "xkT": kT[b], "xvT": vT[b],
            "wqT": np.ascontiguousarray(WqT[:, cols]),
            "wkT": np.ascontiguousarray(WkT[:, cols]),
            "wvT": np.ascontiguousarray(WvT[:, cols]),
            "woT": WoT,
            "bqv": bq[None, cols], "bkv": bk[None, cols],
            "bvv": bv[None, cols], "bov": bo[None, :],
            "onesv": onesr, "onesb": onesbv,
            "mask_pb": m_pb, "maskrep": m_rep, "zmask": zm,
        })

    res = run_bass_kernel_spmd(nc, in_maps, core_ids=list(range(NCORES)),
                               **_cache.get('run_kwargs', {}))
    _cache['last_results'] = res

    full = np.empty((B, S, E), f32)
    for r in range(NCORES):
        b, g = r // G, r % G
        full[b, g * TS:(g + 1) * TS, :] = res.results[r]["out"]
    return full



# revision 19
# speedup vs baseline: 1.3312x; 1.3312x over previous
"""Trainium2 Bass kernel for batched multi-head attention (B=2, S=2048, E=1024, H=16).

Sharding: core r = 4*b + g handles batch b and head-group g (4 heads, 256 emb cols).
- QKV projections: tensor-parallel over head groups; x streamed in 128-row
  contraction blocks, all data bf16 (f32 PSUM accumulation).
- V is projected first and transposed to token-major via DMA-engine xbar
  transposes (no DVE cost), so PV never waits once attention starts.
- Attention: per (head, q-half) stream over 16 key blocks; scores are emitted
  one unit ahead of exp/PV so the Scalar exp stream (the true bottleneck,
  ~1.34us per [128,1024] tile) runs back-to-back and the tensor engine never
  sits behind a stalled PV in queue order.
- Softmax normalization via a mask/ones column appended to V (sumexp lands in
  PSUM row 64); 1/sumexp via single-pass DVE reciprocal_approx_fast.
- Out-proj: token-parallel after per-head 4-rank AllToAll (two replica groups,
  one per batch - no cross-batch zero padding). Out-proj is split in two
  passes: contraction blocks from heads 0/1 are accumulated mid-attention, the
  rest plus bias in a short tail, summed with the stashed partials on DVE.
"""

import sys

if '/opt/trn_rl_repo' not in sys.path:
    sys.path.insert(0, '/opt/trn_rl_repo')

import numpy as np

P = 128
B, S, E, H, DH = 2, 2048, 1024, 16, 64
NCORES = 8
G = 4                 # head groups == cores per batch
EG = E // G           # 256 emb cols per group
TS = S // G           # 512 tokens per core in out-proj
KB = S // P           # 16 key-token blocks
IB = E // P           # 8 contraction blocks of 128
QW = 512              # matmul moving free-dim chunk (one PSUM bank of f32)
HW = 1024             # exp tile q width
SCALE = DH ** -0.5

_cache = {}


def _build():
    import concourse.mybir as mybir
    import concourse.tile as tile
    from concourse import bacc
    from contextlib import ExitStack

    f32 = mybir.dt.float32
    bf16 = mybir.dt.bfloat16
    AF = mybir.ActivationFunctionType

    nc = bacc.Bacc("TRN2", target_bir_lowering=False, debug=False,
                   num_devices=NCORES)

    xqT = nc.dram_tensor("xqT", [E, S], bf16, kind="ExternalInput").ap()
    xkT = nc.dram_tensor("xkT", [E, S], bf16, kind="ExternalInput").ap()
    xvT = nc.dram_tensor("xvT", [E, S], bf16, kind="ExternalInput").ap()
    wqT = nc.dram_tensor("wqT", [E, EG], bf16, kind="ExternalInput").ap()
    wkT = nc.dram_tensor("wkT", [E, EG], bf16, kind="ExternalInput").ap()
    wvT = nc.dram_tensor("wvT", [E, EG], bf16, kind="ExternalInput").ap()
    woT = nc.dram_tensor("woT", [E, E], bf16, kind="ExternalInput").ap()
    bqv = nc.dram_tensor("bqv", [1, EG], bf16, kind="ExternalInput").ap()
    bkv = nc.dram_tensor("bkv", [1, EG], bf16, kind="ExternalInput").ap()
    bvv = nc.dram_tensor("bvv", [1, EG], bf16, kind="ExternalInput").ap()
    bov = nc.dram_tensor("bov", [1, E], bf16, kind="ExternalInput").ap()
    onesb = nc.dram_tensor("onesb", [1, S], bf16, kind="ExternalInput").ap()
    mask_pb = nc.dram_tensor("mask_pb", [P, KB], f32, kind="ExternalInput").ap()
    maskrep = nc.dram_tensor("maskrep", [P, KB * G], f32, kind="ExternalInput").ap()
    zmask = nc.dram_tensor("zmask", [P, 2], f32, kind="ExternalInput").ap()
    out = nc.dram_tensor("out", [TS, E], f32, kind="ExternalOutput").ap()

    a2a_ins = [nc.dram_tensor(f"a2a_in{h}", [NCORES, DH, TS], bf16).ap()
               for h in range(G)]
    a2a_outs = [nc.dram_tensor(f"a2a_out{h}", [NCORES, DH, TS], bf16).ap()
                for h in range(G)]
    groups = [list(range(NCORES))]

    DEBUG = _cache.get('debug', False)
    if DEBUG:
        dbg = {
            "d_vp0": nc.dram_tensor("d_vp0", [P, G * (DH + 1)], bf16,
                                    kind="ExternalOutput").ap(),
            "d_kpT0": nc.dram_tensor("d_kpT0", [P, S], bf16,
                                     kind="ExternalOutput").ap(),
            "d_qpT0": nc.dram_tensor("d_qpT0", [P, S], bf16,
                                     kind="ExternalOutput").ap(),
            "d_cpv": nc.dram_tensor("d_cpv", [DH + 1, HW], f32,
                                    kind="ExternalOutput").ap(),
            "d_rec": nc.dram_tensor("d_rec", [1, HW], f32,
                                    kind="ExternalOutput").ap(),
            "d_ctx": nc.dram_tensor("d_ctx", [DH + 1, HW], bf16,
                                    kind="ExternalOutput").ap(),
            "d_es0": nc.dram_tensor("d_es0", [P, HW], bf16,
                                    kind="ExternalOutput").ap(),
            "d_ga0": nc.dram_tensor("d_ga0", [P, TS], bf16,
                                    kind="ExternalOutput").ap(),
        }

    with tile.TileContext(nc) as tc, ExitStack() as top:
        const = top.enter_context(tc.tile_pool(name="const", bufs=1))

        ones_b = const.tile([1, S], bf16)
        nc.sync.dma_start(ones_b[:], onesb[:])
        bq_r = const.tile([1, EG], bf16)
        nc.sync.dma_start(bq_r[:], bqv[:])
        bk_r = const.tile([1, EG], bf16)
        nc.sync.dma_start(bk_r[:], bkv[:])
        bv_r = const.tile([1, EG], bf16)
        nc.sync.dma_start(bv_r[:], bvv[:])
        bo_r = const.tile([1, E], bf16)
        nc.sync.dma_start(bo_r[:], bov[:])
        mask_t = const.tile([P, KB], f32)
        nc.sync.dma_start(mask_t[:], mask_pb[:])
        maskrep_t = const.tile([P, KB * G], f32)
        nc.sync.dma_start(maskrep_t[:], maskrep[:])
        zmask_t = const.tile([P, 2], f32)
        nc.sync.dma_start(zmask_t[:], zmask[:])

        # persistent SBUF
        proj_sb = top.enter_context(tc.tile_pool(name="proj_sb", bufs=1))
        qpT = [proj_sb.tile([P, S], bf16, tag=f"qpT{m}", name=f"qpT{m}")
               for m in range(2)]
        kpT = [proj_sb.tile([P, S], bf16, tag=f"kpT{m}", name=f"kpT{m}")
               for m in range(2)]
        vpe = [proj_sb.tile([P, S], bf16, tag=f"vpe{m}", name=f"vpe{m}")
               for m in range(2)]
        vp_sb = [proj_sb.tile([P, G * (DH + 1)], bf16, tag=f"vp{j}", name=f"vp{j}")
                 for j in range(KB)]
        ga = [proj_sb.tile([P, TS], bf16, tag=f"ga{ib}", name=f"ga{ib}")
              for ib in range(IB)]
        partial = [proj_sb.tile([P, E], f32, tag=f"part{tm}", name=f"part{tm}")
                   for tm in range(TS // P)]

        # projection + out-proj weights
        w_pool = top.enter_context(tc.tile_pool(name="wqkv", bufs=1))
        w_r = {}
        for name, wap in (("v", wvT), ("k", wkT), ("q", wqT)):
            wr = w_pool.tile([P, IB * EG], bf16, tag=f"w{name}r", name=f"w{name}r")
            nc.sync.dma_start(wr[:].rearrange("p (i e) -> p i e", e=EG),
                              wap.rearrange("(i p) e -> p i e", p=P))
            w_r[name] = wr
        wo_r = w_pool.tile([P, IB * E], bf16, tag="wor", name="wor")

        # ---- projections: V first (vp feeds PV), then K, then Q ----
        with tc.tile_pool(name="xst", bufs=3) as xst, \
             tc.tile_pool(name="kqpsum", bufs=1, space="PSUM") as kqpsum:
            for name, xap, brow in (("v", xvT, bv_r), ("k", xkT, bk_r),
                                    ("q", xqT, bq_r)):
                kqs = [kqpsum.tile([P, S], f32, tag=f"kqs{m}", name=f"kqs{m}")
                       for m in range(2)]
                for i in range(IB):
                    xr = xst.tile([P, S], bf16)
                    nc.sync.dma_start(xr[:], xap[i * P:(i + 1) * P, :])
                    for m in range(2):
                        for c in range(S // QW):
                            nc.tensor.matmul(
                                kqs[m][:, c * QW:(c + 1) * QW],
                                w_r[name][:, i * EG + m * P:i * EG + (m + 1) * P],
                                xr[:, c * QW:(c + 1) * QW],
                                start=(i == 0), stop=False)
                for m in range(2):
                    for c in range(S // QW):
                        nc.tensor.matmul(
                            kqs[m][:, c * QW:(c + 1) * QW],
                            brow[0:1, m * P:(m + 1) * P],
                            ones_b[0:1, c * QW:(c + 1) * QW],
                            start=False, stop=True)
                dsts = {"v": vpe, "k": kpT, "q": qpT}[name]
                for m in range(2):
                    nc.vector.tensor_copy(dsts[m][:], kqs[m][:])
                if name == "v":
                    # token-major vp tiles via DMA xbar transpose (contiguous
                    # dst only — strided xbar dst scrambles), then DVE copy
                    # into the head-block layout. mask/ones column sits at
                    # slot 0 of each head block so sumexp lands in PSUM
                    # partition 0 (reciprocal_approx_fast needs partition 0).
                    with tc.tile_pool(name="tokp", bufs=4) as tokp:
                        for j in range(KB):
                            for m in range(2):
                                tok = tokp.tile([P, P], bf16)
                                nc.sync.dma_start_transpose(
                                    tok[:], vpe[m][:, j * P:(j + 1) * P])
                                dst3 = vp_sb[j].rearrange(
                                    "p (h e) -> p h e",
                                    e=DH + 1)[:, 2 * m:2 * m + 2, 1:DH + 1]
                                nc.vector.tensor_copy(
                                    dst3, tok.rearrange("p (h e) -> p h e", e=DH))
                            nc.vector.tensor_copy(
                                vp_sb[j].rearrange(
                                    "p (h e) -> p h e", e=DH + 1)[:, :, 0:1],
                                maskrep_t[:, j * G:(j + 1) * G].rearrange(
                                    "p (h e) -> p h e", e=1))
                            vals = vp_sb[j].rearrange(
                                "p (h e) -> p h e", e=DH + 1)[:, :, 1:DH + 1]
                            nc.vector.tensor_scalar_mul(vals, vals,
                                                        mask_t[:, j:j + 1])

        # out-proj weights: emitted after x so the queues drain x first
        nc.sync.dma_start(wo_r[:].rearrange("p (i e) -> p i e", e=E),
                          woT.rearrange("(i p) e -> p i e", p=P))

        # ---- attention: flat pipelined stream over (head, q-half, kblock) ----
        flat = [(h, half, j)
                for h in range(G) for half in range(2) for j in range(KB)]

        with tc.tile_pool(name="spsum", bufs=2, space="PSUM") as spsum, \
             tc.tile_pool(name="pvpsum", bufs=1, space="PSUM") as pvpsum, \
             tc.tile_pool(name="opsum", bufs=1, space="PSUM") as opsum, \
             tc.tile_pool(name="expp", bufs=10) as expp, \
             tc.tile_pool(name="cpvp", bufs=2) as cpvp, \
             tc.tile_pool(name="normp", bufs=2) as normp, \
             tc.tile_pool(name="ctxp", bufs=2) as ctxp, \
             tc.tile_pool(name="foutp", bufs=2) as foutp:

            sp_t = {}
            es_t = {}
            pv_t = {}
            oproj_chunks = []

            def emit_scores(u):
                h, half, j = flat[u]
                mt, po = h // 2, (h % 2) * DH
                sp = spsum.tile([P, HW], f32, tag="sp", name=f"sp{u}")
                for c in range(HW // QW):
                    q0 = half * HW + c * QW
                    nc.tensor.matmul(
                        sp[:, c * QW:(c + 1) * QW],
                        kpT[mt][po:po + DH, j * P:(j + 1) * P],
                        qpT[mt][po:po + DH, q0:q0 + QW],
                        start=True, stop=True)
                sp_t[u] = sp

            def emit_exp(u):
                es = expp.tile([P, HW], bf16)
                nc.scalar.activation(es[:], sp_t.pop(u)[:], AF.Exp, scale=SCALE)
                es_t[u] = es
                if DEBUG and u == 0:
                    nc.sync.dma_start(dbg["d_es0"][:], es[:])

            def emit_pv(u):
                h, half, j = flat[u]
                if j == 0:
                    pv_t[(h, half)] = pvpsum.tile([DH + 1, HW], f32, tag="pv",
                                                  name=f"pv{h}_{half}")
                pv = pv_t[(h, half)]
                es = es_t.pop(u)
                for c in range(HW // QW):
                    nc.tensor.matmul(
                        pv[:, c * QW:(c + 1) * QW],
                        vp_sb[j][:, h * (DH + 1):(h + 1) * (DH + 1)],
                        es[:, c * QW:(c + 1) * QW],
                        start=(j == 0), stop=(j == KB - 1))
                if j == KB - 1:
                    emit_norm_send(h, half)

            def emit_norm_send(h, half):
                # copy pv out of PSUM first so the single pv buffer frees fast
                pv = pv_t.pop((h, half))
                cpv = cpvp.tile([DH + 1, HW], f32)
                nc.vector.tensor_copy(cpv[:], pv[:])
                rec = normp.tile([1, HW], f32, tag="rec", name="rec")
                nc.vector.reciprocal_approx_fast(rec[:], cpv[0:1, :])
                recB = normp.tile([DH + 1, HW], f32, tag="recB", name="recB")
                nc.gpsimd.partition_broadcast(recB[:], rec[:])
                # row 0 becomes sumexp/sumexp ~ 1; rows 1..64 the context
                ctx = ctxp.tile([DH + 1, HW], bf16)
                nc.vector.tensor_mul(ctx[:], cpv[:], recB[:])
                # slot pairs (tsl, tsl+4): exactly one is this batch's rank;
                # the host 0/1 zmask zeroes the other so the 8-rank AllToAll
                # stays SPMD-uniform
                ctxA = ctxp.tile([DH + 1, HW], bf16, tag="ctxA", name="ctxA")
                nc.vector.tensor_scalar_mul(ctxA[:], ctx[:], zmask_t[0:DH + 1, 0:1])
                ctxB = ctxp.tile([DH + 1, HW], bf16, tag="ctxB", name="ctxB")
                nc.vector.tensor_scalar_mul(ctxB[:], ctx[:], zmask_t[0:DH + 1, 1:2])
                if DEBUG and (h, half) == (0, 0):
                    nc.sync.dma_start(dbg["d_cpv"][:], cpv[:])
                    nc.sync.dma_start(dbg["d_rec"][:], rec[:])
                    nc.sync.dma_start(dbg["d_ctx"][:], ctx[:])
                for jj in range(2):
                    tsl = half * 2 + jj
                    nc.sync.dma_start(a2a_ins[h][tsl],
                                      ctxA[1:DH + 1, jj * TS:(jj + 1) * TS])
                    nc.sync.dma_start(a2a_ins[h][tsl + 4],
                                      ctxB[1:DH + 1, jj * TS:(jj + 1) * TS])
                if half == 1:
                    nc.gpsimd.collective_compute(
                        "AllToAll", mybir.AluOpType.bypass,
                        replica_groups=groups,
                        ins=[a2a_ins[h][:]], outs=[a2a_outs[h][:]])
                    m, r0 = h // 2, (h % 2) * DH
                    for gp in range(G):
                        gs0 = ctxp.tile([DH, TS], bf16, tag="gs0", name="gs0")
                        gs1 = ctxp.tile([DH, TS], bf16, tag="gs1", name="gs1")
                        nc.sync.dma_start(gs0[:], a2a_outs[h][gp])
                        nc.sync.dma_start(gs1[:], a2a_outs[h][gp + 4])
                        nc.vector.tensor_add(ga[gp * 2 + m][r0:r0 + DH, :],
                                             gs0[:], gs1[:])

            IBS1 = [0, 2, 4, 6]   # contraction blocks from heads 0,1

            def emit_oproj_chunk(tm, n):
                # pass 1, one (tm, ib) chunk: 2 matmuls, interleaved into the
                # attention stream so the in-order tensor queue never parks
                # behind a collective wait
                ib = IBS1[n]
                pot = opsum.tile([P, E], f32, tag="pot", name=f"pot{tm}") \
                    if n == 0 else oproj_pot[0]
                if n == 0:
                    oproj_pot[0] = pot
                for oc in range(E // QW):
                    nc.tensor.matmul(
                        pot[:, oc * QW:(oc + 1) * QW],
                        ga[ib][:, tm * P:(tm + 1) * P],
                        wo_r[:, ib * E + oc * QW:ib * E + oc * QW + QW],
                        start=(n == 0), stop=(n == len(IBS1) - 1))
                if n == len(IBS1) - 1:
                    nc.vector.tensor_copy(partial[tm][:], pot[:])

            oproj_pot = [None]

            def emit_oproj_pass2():
                for tm in range(TS // P):
                    pot = opsum.tile([P, E], f32, tag="pot", name=f"pot2_{tm}")
                    for n, ib in enumerate([1, 3, 5, 7]):
                        for oc in range(E // QW):
                            nc.tensor.matmul(
                                pot[:, oc * QW:(oc + 1) * QW],
                                ga[ib][:, tm * P:(tm + 1) * P],
                                wo_r[:, ib * E + oc * QW:ib * E + oc * QW + QW],
                                start=(n == 0), stop=False)
                    for oc in range(E // QW):
                        nc.tensor.matmul(
                            pot[:, oc * QW:(oc + 1) * QW],
                            ones_b[0:1, 0:P],
                            bo_r[0:1, oc * QW:(oc + 1) * QW],
                            start=False, stop=True)
                    fout = foutp.tile([P, E], f32)
                    nc.vector.tensor_add(fout[:], pot[:], partial[tm][:])
                    nc.sync.dma_start(out[tm * P:(tm + 1) * P, :], fout[:])

            # pass-1 chunks start 8 units after head 1's AllToAll fires
            OPROJ_U0 = 72
            oproj_chunks = [(tm, n) for tm in range(TS // P)
                            for n in range(len(IBS1))]

            emit_scores(0)
            for u in range(len(flat)):
                if u + 1 < len(flat):
                    emit_scores(u + 1)
                emit_exp(u)
                emit_pv(u)
                if u >= OPROJ_U0 and oproj_chunks:
                    emit_oproj_chunk(*oproj_chunks.pop(0))
            while oproj_chunks:
                emit_oproj_chunk(*oproj_chunks.pop(0))
            emit_oproj_pass2()
            if DEBUG:
                nc.sync.dma_start(dbg["d_vp0"][:], vp_sb[0][:])
                nc.sync.dma_start(dbg["d_kpT0"][:], kpT[0][:])
                nc.sync.dma_start(dbg["d_qpT0"][:], qpT[0][:])
                nc.sync.dma_start(dbg["d_ga0"][:], ga[0][:])

    nc.compile()
    return nc


def _get_nc():
    if 'nc' not in _cache:
        _cache['nc'] = _build()
    return _cache['nc']


def kernel(q, k, v, mask, Wq, bq, Wk, bk, Wv, bv, Wo, bo):
    import ml_dtypes
    from concourse.bass_utils import run_bass_kernel_spmd

    nc = _get_nc()
    bft = ml_dtypes.bfloat16
    f32 = np.float32

    def _cvt(x):
        return np.ascontiguousarray(np.asarray(x, f32)).astype(bft)

    q = np.asarray(q, f32)
    k = np.asarray(k, f32)
    v = np.asarray(v, f32)
    qT = [_cvt(q[b].T) for b in range(B)]
    kT = [_cvt(k[b].T) for b in range(B)]
    vT = [_cvt(v[b].T) for b in range(B)]
    WqT = _cvt(np.asarray(Wq, f32).T)
    WkT = _cvt(np.asarray(Wk, f32).T)
    WvT = _cvt(np.asarray(Wv, f32).T)
    WoT = _cvt(np.asarray(Wo, f32).T)
    bq = _cvt(bq)
    bk = _cvt(bk)
    bv = _cvt(bv)
    bo = _cvt(bo)
    onesbv = np.ones((1, S), bft)
    maskf = (np.asarray(mask) != 0).astype(f32)  # [B, S]

    in_maps = []
    for r in range(NCORES):
        b, g = r // G, r % G
        cols = slice(g * EG, (g + 1) * EG)
        m_pb = np.ascontiguousarray(maskf[b].reshape(KB, P).T)       # [128,16]
        m_rep = np.ascontiguousarray(np.repeat(m_pb, G, axis=1))     # [128,64]
        zm = np.zeros((P, 2), f32)
        zm[:, b] = 1.0
        in_maps.append({
            "xqT": qT[b], "xkT": kT[b], "xvT": vT[b],
            "wqT": np.ascontiguousarray(WqT[:, cols]),
            "wkT": np.ascontiguousarray(WkT[:, cols]),
            "wvT": np.ascontiguousarray(WvT[:, cols]),
            "woT": WoT,
            "bqv": bq[None, cols], "bkv": bk[None, cols],
            "bvv": bv[None, cols], "bov": bo[None, :],
            "onesb": onesbv,
            "mask_pb": m_pb, "maskrep": m_rep, "zmask": zm,
        })

    res = run_bass_kernel_spmd(nc, in_maps, core_ids=list(range(NCORES)),
                               **_cache.get('run_kwargs', {}))
    _cache['last_results'] = res

    full = np.empty((B, S, E), f32)
    for r in range(NCORES):
        b, g = r // G, r % G
        full[b, g * TS:(g + 1) * TS, :] = res.results[r]["out"]
    return full


# revision 22
# speedup vs baseline: 1.4563x; 1.0940x over previous
"""Trainium2 Bass kernel for batched multi-head attention (B=2, S=2048, E=1024, H=16).

Sharding: core r = 4*b + g handles batch b and head-group g (4 heads, 256 emb cols).
- QKV projections: tensor-parallel over head groups; x streamed in 128-row
  contraction blocks, all data bf16 (f32 PSUM accumulation).
- V is projected first and transposed to token-major via DMA-engine xbar
  transposes (no DVE cost), so PV never waits once attention starts.
- Attention: per (head, q-half) stream over 16 key blocks; scores are emitted
  one unit ahead of exp/PV so the Scalar exp stream (the true bottleneck,
  ~1.34us per [128,1024] tile) runs back-to-back and the tensor engine never
  sits behind a stalled PV in queue order.
- Softmax normalization via a mask/ones column appended to V (sumexp lands in
  PSUM row 64); 1/sumexp via single-pass DVE reciprocal_approx_fast.
- Out-proj: token-parallel after per-head 4-rank AllToAll (two replica groups,
  one per batch - no cross-batch zero padding). Out-proj is split in two
  passes: contraction blocks from heads 0/1 are accumulated mid-attention, the
  rest plus bias in a short tail, summed with the stashed partials on DVE.
"""

import sys

if '/opt/trn_rl_repo' not in sys.path:
    sys.path.insert(0, '/opt/trn_rl_repo')

import numpy as np

P = 128
B, S, E, H, DH = 2, 2048, 1024, 16, 64
NCORES = 8
G = 4                 # head groups == cores per batch
EG = E // G           # 256 emb cols per group
TS = S // G           # 512 tokens per core in out-proj
KB = S // P           # 16 key-token blocks
IB = E // P           # 8 contraction blocks of 128
QW = 512              # matmul moving free-dim chunk (one PSUM bank of f32)
HW = 1024             # exp tile q width
SCALE = DH ** -0.5

_cache = {}


def _build():
    import concourse.mybir as mybir
    import concourse.tile as tile
    from concourse import bacc
    from contextlib import ExitStack

    f32 = mybir.dt.float32
    bf16 = mybir.dt.bfloat16
    AF = mybir.ActivationFunctionType

    nc = bacc.Bacc("TRN2", target_bir_lowering=False, debug=False,
                   num_devices=NCORES)

    xqT = nc.dram_tensor("xqT", [E, S], bf16, kind="ExternalInput").ap()
    xkT = nc.dram_tensor("xkT", [E, S], bf16, kind="ExternalInput").ap()
    xvT = nc.dram_tensor("xvT", [E, S], bf16, kind="ExternalInput").ap()
    wqT = nc.dram_tensor("wqT", [E, EG], bf16, kind="ExternalInput").ap()
    wkT = nc.dram_tensor("wkT", [E, EG], bf16, kind="ExternalInput").ap()
    wvT = nc.dram_tensor("wvT", [E, EG], bf16, kind="ExternalInput").ap()
    woT = nc.dram_tensor("woT", [E, E], bf16, kind="ExternalInput").ap()
    bqv = nc.dram_tensor("bqv", [1, EG], bf16, kind="ExternalInput").ap()
    bkv = nc.dram_tensor("bkv", [1, EG], bf16, kind="ExternalInput").ap()
    bvv = nc.dram_tensor("bvv", [1, EG], bf16, kind="ExternalInput").ap()
    bov = nc.dram_tensor("bov", [1, E], bf16, kind="ExternalInput").ap()
    onesb = nc.dram_tensor("onesb", [1, S], bf16, kind="ExternalInput").ap()
    mask_pb = nc.dram_tensor("mask_pb", [P, KB], f32, kind="ExternalInput").ap()
    maskrep = nc.dram_tensor("maskrep", [P, KB * G], f32, kind="ExternalInput").ap()
    zmask = nc.dram_tensor("zmask", [P, 2], f32, kind="ExternalInput").ap()
    out = nc.dram_tensor("out", [TS, E], f32, kind="ExternalOutput").ap()

    a2a_ins = [nc.dram_tensor(f"a2a_in{h}", [NCORES, DH, TS], bf16).ap()
               for h in range(G)]
    a2a_outs = [nc.dram_tensor(f"a2a_out{h}", [NCORES, DH, TS], bf16).ap()
                for h in range(G)]
    groups = [list(range(NCORES))]

    DEBUG = _cache.get('debug', False)
    if DEBUG:
        dbg = {
            "d_vp0": nc.dram_tensor("d_vp0", [P, G * (DH + 1)], bf16,
                                    kind="ExternalOutput").ap(),
            "d_kpT0": nc.dram_tensor("d_kpT0", [P, S], bf16,
                                     kind="ExternalOutput").ap(),
            "d_qpT0": nc.dram_tensor("d_qpT0", [P, S], bf16,
                                     kind="ExternalOutput").ap(),
            "d_cpv": nc.dram_tensor("d_cpv", [DH + 1, HW], f32,
                                    kind="ExternalOutput").ap(),
            "d_rec": nc.dram_tensor("d_rec", [1, HW], f32,
                                    kind="ExternalOutput").ap(),
            "d_ctx": nc.dram_tensor("d_ctx", [DH + 1, HW], bf16,
                                    kind="ExternalOutput").ap(),
            "d_es0": nc.dram_tensor("d_es0", [P, HW], bf16,
                                    kind="ExternalOutput").ap(),
            "d_ga0": nc.dram_tensor("d_ga0", [P, TS], bf16,
                                    kind="ExternalOutput").ap(),
        }

    with tile.TileContext(nc) as tc, ExitStack() as top:
        const = top.enter_context(tc.tile_pool(name="const", bufs=1))

        ones_b = const.tile([1, S], bf16)
        nc.sync.dma_start(ones_b[:], onesb[:])
        bq_r = const.tile([1, EG], bf16)
        nc.sync.dma_start(bq_r[:], bqv[:])
        bk_r = const.tile([1, EG], bf16)
        nc.sync.dma_start(bk_r[:], bkv[:])
        bv_r = const.tile([1, EG], bf16)
        nc.sync.dma_start(bv_r[:], bvv[:])
        bo_r = const.tile([1, E], bf16)
        nc.sync.dma_start(bo_r[:], bov[:])
        mask_t = const.tile([P, KB], f32)
        nc.sync.dma_start(mask_t[:], mask_pb[:])
        maskrep_t = const.tile([P, KB * G], f32)
        nc.sync.dma_start(maskrep_t[:], maskrep[:])
        zmask_t = const.tile([P, 2], f32)
        nc.sync.dma_start(zmask_t[:], zmask[:])

        # persistent SBUF
        proj_sb = top.enter_context(tc.tile_pool(name="proj_sb", bufs=1))
        qpT = [proj_sb.tile([P, S], bf16, tag=f"qpT{m}", name=f"qpT{m}")
               for m in range(2)]
        kpT = [proj_sb.tile([P, S], bf16, tag=f"kpT{m}", name=f"kpT{m}")
               for m in range(2)]
        vpe = [proj_sb.tile([P, S], bf16, tag=f"vpe{m}", name=f"vpe{m}")
               for m in range(2)]
        vp_sb = [proj_sb.tile([P, G * (DH + 1)], bf16, tag=f"vp{j}", name=f"vp{j}")
                 for j in range(KB)]
        ga = [proj_sb.tile([P, TS], bf16, tag=f"ga{ib}", name=f"ga{ib}")
              for ib in range(IB)]
        partial = [proj_sb.tile([P, E], f32, tag=f"part{tm}", name=f"part{tm}")
                   for tm in range(TS // P)]

        # projection + out-proj weights
        w_pool = top.enter_context(tc.tile_pool(name="wqkv", bufs=1))
        w_r = {}
        for name, wap in (("v", wvT), ("k", wkT), ("q", wqT)):
            wr = w_pool.tile([P, IB * EG], bf16, tag=f"w{name}r", name=f"w{name}r")
            nc.sync.dma_start(wr[:].rearrange("p (i e) -> p i e", e=EG),
                              wap.rearrange("(i p) e -> p i e", p=P))
            w_r[name] = wr
        wo_r = w_pool.tile([P, IB * E], bf16, tag="wor", name="wor")

        # ---- projections: V first (vp feeds PV), then K, then Q ----
        with tc.tile_pool(name="xst", bufs=3) as xst, \
             tc.tile_pool(name="kqpsum", bufs=1, space="PSUM") as kqpsum:
            for name, xap, brow in (("v", xvT, bv_r), ("k", xkT, bk_r),
                                    ("q", xqT, bq_r)):
                kqs = [kqpsum.tile([P, S], f32, tag=f"kqs{m}", name=f"kqs{m}")
                       for m in range(2)]
                for i in range(IB):
                    xr = xst.tile([P, S], bf16)
                    nc.sync.dma_start(xr[:], xap[i * P:(i + 1) * P, :])
                    for m in range(2):
                        for c in range(S // QW):
                            nc.tensor.matmul(
                                kqs[m][:, c * QW:(c + 1) * QW],
                                w_r[name][:, i * EG + m * P:i * EG + (m + 1) * P],
                                xr[:, c * QW:(c + 1) * QW],
                                start=(i == 0), stop=False)
                for m in range(2):
                    for c in range(S // QW):
                        nc.tensor.matmul(
                            kqs[m][:, c * QW:(c + 1) * QW],
                            brow[0:1, m * P:(m + 1) * P],
                            ones_b[0:1, c * QW:(c + 1) * QW],
                            start=False, stop=True)
                dsts = {"v": vpe, "k": kpT, "q": qpT}[name]
                for m in range(2):
                    nc.vector.tensor_copy(dsts[m][:], kqs[m][:])

        # out-proj weights: emitted after x so the queues drain x first
        nc.sync.dma_start(wo_r[:].rearrange("p (i e) -> p i e", e=E),
                          woT.rearrange("(i p) e -> p i e", p=P))

        # token-major vp tiles via DMA xbar transpose (contiguous dst only —
        # strided xbar dst scrambles), then DVE copy into the head-block
        # layout. Emitted AFTER all x loads: the transposes wait on vpe in
        # the in-order SP queue, and anything queued behind them would stall.
        # mask/ones column sits at slot 0 of each head block so sumexp lands
        # in PSUM partition 0 (reciprocal_approx_fast needs partition 0).
        tokp = top.enter_context(tc.tile_pool(name="tokp", bufs=4))
        for j in range(KB):
            for m in range(2):
                tok = tokp.tile([P, P], bf16)
                nc.sync.dma_start_transpose(
                    tok[:], vpe[m][:, j * P:(j + 1) * P])
                dst3 = vp_sb[j].rearrange(
                    "p (h e) -> p h e", e=DH + 1)[:, 2 * m:2 * m + 2, 1:DH + 1]
                nc.vector.tensor_copy(
                    dst3, tok.rearrange("p (h e) -> p h e", e=DH))
            nc.vector.tensor_copy(
                vp_sb[j].rearrange("p (h e) -> p h e", e=DH + 1)[:, :, 0:1],
                maskrep_t[:, j * G:(j + 1) * G].rearrange(
                    "p (h e) -> p h e", e=1))
            vals = vp_sb[j].rearrange(
                "p (h e) -> p h e", e=DH + 1)[:, :, 1:DH + 1]
            nc.vector.tensor_scalar_mul(vals, vals, mask_t[:, j:j + 1])

        # ---- attention: flat pipelined stream over (head, q-half, kblock) ----
        flat = [(h, half, j)
                for h in range(G) for half in range(2) for j in range(KB)]

        with tc.tile_pool(name="spsum", bufs=2, space="PSUM") as spsum, \
             tc.tile_pool(name="pvpsum", bufs=1, space="PSUM") as pvpsum, \
             tc.tile_pool(name="opsum", bufs=1, space="PSUM") as opsum, \
             tc.tile_pool(name="expp", bufs=10) as expp, \
             tc.tile_pool(name="cpvp", bufs=2) as cpvp, \
             tc.tile_pool(name="normp", bufs=2) as normp, \
             tc.tile_pool(name="ctxp", bufs=2) as ctxp, \
             tc.tile_pool(name="foutp", bufs=2) as foutp:

            sp_t = {}
            es_t = {}
            pv_t = {}
            oproj_chunks = []

            def emit_scores(u):
                h, half, j = flat[u]
                mt, po = h // 2, (h % 2) * DH
                sp = spsum.tile([P, HW], f32, tag="sp", name=f"sp{u}")
                for c in range(HW // QW):
                    q0 = half * HW + c * QW
                    nc.tensor.matmul(
                        sp[:, c * QW:(c + 1) * QW],
                        kpT[mt][po:po + DH, j * P:(j + 1) * P],
                        qpT[mt][po:po + DH, q0:q0 + QW],
                        start=True, stop=True)
                sp_t[u] = sp

            def emit_exp(u):
                es = expp.tile([P, HW], bf16)
                nc.scalar.activation(es[:], sp_t.pop(u)[:], AF.Exp, scale=SCALE)
                es_t[u] = es
                if DEBUG and u == 0:
                    nc.sync.dma_start(dbg["d_es0"][:], es[:])

            def emit_pv(u):
                h, half, j = flat[u]
                if j == 0:
                    pv_t[(h, half)] = pvpsum.tile([DH + 1, HW], f32, tag="pv",
                                                  name=f"pv{h}_{half}")
                pv = pv_t[(h, half)]
                es = es_t.pop(u)
                for c in range(HW // QW):
                    nc.tensor.matmul(
                        pv[:, c * QW:(c + 1) * QW],
                        vp_sb[j][:, h * (DH + 1):(h + 1) * (DH + 1)],
                        es[:, c * QW:(c + 1) * QW],
                        start=(j == 0), stop=(j == KB - 1))
                if j == KB - 1:
                    emit_norm_send(h, half)

            def emit_norm_send(h, half):
                # copy pv out of PSUM first so the single pv buffer frees fast
                pv = pv_t.pop((h, half))
                cpv = cpvp.tile([DH + 1, HW], f32)
                nc.vector.tensor_copy(cpv[:], pv[:])
                rec = normp.tile([1, HW], f32, tag="rec", name="rec")
                nc.vector.reciprocal_approx_fast(rec[:], cpv[0:1, :])
                recB = normp.tile([DH + 1, HW], f32, tag="recB", name="recB")
                nc.gpsimd.partition_broadcast(recB[:], rec[:])
                # row 0 becomes sumexp/sumexp ~ 1; rows 1..64 the context
                ctx = ctxp.tile([DH + 1, HW], bf16)
                nc.vector.tensor_mul(ctx[:], cpv[:], recB[:])
                # slot pairs (tsl, tsl+4): exactly one is this batch's rank;
                # the host 0/1 zmask zeroes the other so the 8-rank AllToAll
                # stays SPMD-uniform
                ctxA = ctxp.tile([DH + 1, HW], bf16, tag="ctxA", name="ctxA")
                nc.vector.tensor_scalar_mul(ctxA[:], ctx[:], zmask_t[0:DH + 1, 0:1])
                ctxB = ctxp.tile([DH + 1, HW], bf16, tag="ctxB", name="ctxB")
                nc.vector.tensor_scalar_mul(ctxB[:], ctx[:], zmask_t[0:DH + 1, 1:2])
                if DEBUG and (h, half) == (0, 0):
                    nc.sync.dma_start(dbg["d_cpv"][:], cpv[:])
                    nc.sync.dma_start(dbg["d_rec"][:], rec[:])
                    nc.sync.dma_start(dbg["d_ctx"][:], ctx[:])
                for jj in range(2):
                    tsl = half * 2 + jj
                    nc.sync.dma_start(a2a_ins[h][tsl],
                                      ctxA[1:DH + 1, jj * TS:(jj + 1) * TS])
                    nc.sync.dma_start(a2a_ins[h][tsl + 4],
                                      ctxB[1:DH + 1, jj * TS:(jj + 1) * TS])
                if half == 1:
                    nc.gpsimd.collective_compute(
                        "AllToAll", mybir.AluOpType.bypass,
                        replica_groups=groups,
                        ins=[a2a_ins[h][:]], outs=[a2a_outs[h][:]])
                    m, r0 = h // 2, (h % 2) * DH
                    for gp in range(G):
                        gs0 = ctxp.tile([DH, TS], bf16, tag="gs0", name="gs0")
                        gs1 = ctxp.tile([DH, TS], bf16, tag="gs1", name="gs1")
                        nc.sync.dma_start(gs0[:], a2a_outs[h][gp])
                        nc.sync.dma_start(gs1[:], a2a_outs[h][gp + 4])
                        nc.vector.tensor_add(ga[gp * 2 + m][r0:r0 + DH, :],
                                             gs0[:], gs1[:])

            IBS1 = [0, 2, 4, 6]   # contraction blocks from heads 0,1

            def emit_oproj_chunk(tm, n, oc):
                # pass 1, one matmul at a time, interleaved into the attention
                # stream so the in-order tensor queue never parks behind a
                # collective wait and never starves the exp stream
                ib = IBS1[n]
                if n == 0 and oc == 0:
                    oproj_pot[0] = opsum.tile([P, E], f32, tag="pot",
                                              name=f"pot{tm}")
                pot = oproj_pot[0]
                nc.tensor.matmul(
                    pot[:, oc * QW:(oc + 1) * QW],
                    ga[ib][:, tm * P:(tm + 1) * P],
                    wo_r[:, ib * E + oc * QW:ib * E + oc * QW + QW],
                    start=(n == 0), stop=(n == len(IBS1) - 1))
                if n == len(IBS1) - 1 and oc == E // QW - 1:
                    nc.vector.tensor_copy(partial[tm][:], pot[:])

            oproj_pot = [None]

            def emit_oproj_pass2():
                for tm in range(TS // P):
                    pot = opsum.tile([P, E], f32, tag="pot", name=f"pot2_{tm}")
                    for n, ib in enumerate([1, 3, 5, 7]):
                        for oc in range(E // QW):
                            nc.tensor.matmul(
                                pot[:, oc * QW:(oc + 1) * QW],
                                ga[ib][:, tm * P:(tm + 1) * P],
                                wo_r[:, ib * E + oc * QW:ib * E + oc * QW + QW],
                                start=(n == 0), stop=False)
                    for oc in range(E // QW):
                        nc.tensor.matmul(
                            pot[:, oc * QW:(oc + 1) * QW],
                            ones_b[0:1, 0:P],
                            bo_r[0:1, oc * QW:(oc + 1) * QW],
                            start=False, stop=True)
                    fout = foutp.tile([P, E], f32)
                    nc.vector.tensor_add(fout[:], pot[:], partial[tm][:])
                    nc.sync.dma_start(out[tm * P:(tm + 1) * P, :], fout[:])

            # PV lags the exp stream so tensor work comes in contiguous
            # bursts (scores u+1 then pv u-LAG back to back, no short waits)
            PV_LAG = 4
            # pass-1 chunks start well after head 1's AllToAll completes
            OPROJ_U0 = 92
            oproj_chunks = [(tm, n, oc) for tm in range(TS // P)
                            for n in range(len(IBS1)) for oc in range(E // QW)]

            emit_scores(0)
            for u in range(len(flat)):
                if u + 1 < len(flat):
                    emit_scores(u + 1)
                emit_exp(u)
                if u >= PV_LAG:
                    emit_pv(u - PV_LAG)
                if u >= OPROJ_U0 and oproj_chunks:
                    emit_oproj_chunk(*oproj_chunks.pop(0))
            for u in range(len(flat) - PV_LAG, len(flat)):
                emit_pv(u)
            while oproj_chunks:
                emit_oproj_chunk(*oproj_chunks.pop(0))
            emit_oproj_pass2()
            if DEBUG:
                nc.sync.dma_start(dbg["d_vp0"][:], vp_sb[0][:])
                nc.sync.dma_start(dbg["d_kpT0"][:], kpT[0][:])
                nc.sync.dma_start(dbg["d_qpT0"][:], qpT[0][:])
                nc.sync.dma_start(dbg["d_ga0"][:], ga[0][:])

    nc.compile()
    return nc


def _get_nc():
    if 'nc' not in _cache:
        _cache['nc'] = _build()
    return _cache['nc']


def kernel(q, k, v, mask, Wq, bq, Wk, bk, Wv, bv, Wo, bo):
    import ml_dtypes
    from concourse.bass_utils import run_bass_kernel_spmd

    nc = _get_nc()
    bft = ml_dtypes.bfloat16
    f32 = np.float32

    def _cvt(x):
        return np.ascontiguousarray(np.asarray(x, f32)).astype(bft)

    q = np.asarray(q, f32)
    k = np.asarray(k, f32)
    v = np.asarray(v, f32)
    qT = [_cvt(q[b].T) for b in range(B)]
    kT = [_cvt(k[b].T) for b in range(B)]
    vT = [_cvt(v[b].T) for b in range(B)]
    WqT = _cvt(np.asarray(Wq, f32).T)
    WkT = _cvt(np.asarray(Wk, f32).T)
    WvT = _cvt(np.asarray(Wv, f32).T)
    WoT = _cvt(np.asarray(Wo, f32).T)
    bq = _cvt(bq)
    bk = _cvt(bk)
    bv = _cvt(bv)
    bo = _cvt(bo)
    onesbv = np.ones((1, S), bft)
    maskf = (np.asarray(mask) != 0).astype(f32)  # [B, S]

    in_maps = []
    for r in range(NCORES):
        b, g = r // G, r % G
        cols = slice(g * EG, (g + 1) * EG)
        m_pb = np.ascontiguousarray(maskf[b].reshape(KB, P).T)       # [128,16]
        m_rep = np.ascontiguousarray(np.repeat(m_pb, G, axis=1))     # [128,64]
        zm = np.zeros((P, 2), f32)
        zm[:, b] = 1.0
        in_maps.append({
            "xqT": qT[b], "xkT": kT[b], "xvT": vT[b],
            "wqT": np.ascontiguousarray(WqT[:, cols]),
            "wkT": np.ascontiguousarray(WkT[:, cols]),
            "wvT": np.ascontiguousarray(WvT[:, cols]),
            "woT": WoT,
            "bqv": bq[None, cols], "bkv": bk[None, cols],
            "bvv": bv[None, cols], "bov": bo[None, :],
            "onesb": onesbv,
            "mask_pb": m_pb, "maskrep": m_rep, "zmask": zm,
        })

    res = run_bass_kernel_spmd(nc, in_maps, core_ids=list(range(NCORES)),
                               **_cache.get('run_kwargs', {}))
    _cache['last_results'] = res

    full = np.empty((B, S, E), f32)
    for r in range(NCORES):
        b, g = r // G, r % G
        full[b, g * TS:(g + 1) * TS, :] = res.results[r]["out"]
    return full


# revision 25
# speedup vs baseline: 1.4928x; 1.0250x over previous
"""Trainium2 Bass kernel for batched multi-head attention (B=2, S=2048, E=1024, H=16).

Sharding: core r = 4*b + g handles batch b and head-group g (4 heads, 256 emb cols).
- QKV projections: tensor-parallel over head groups; x streamed in 128-row
  contraction blocks, all data bf16 (f32 PSUM accumulation).
- V is projected first and transposed to token-major via DMA-engine xbar
  transposes (no DVE cost), so PV never waits once attention starts.
- Attention: per (head, q-half) stream over 16 key blocks; scores are emitted
  one unit ahead of exp/PV so the Scalar exp stream (the true bottleneck,
  ~1.34us per [128,1024] tile) runs back-to-back and the tensor engine never
  sits behind a stalled PV in queue order.
- Softmax normalization via a mask/ones column appended to V (sumexp lands in
  PSUM row 64); 1/sumexp via single-pass DVE reciprocal_approx_fast.
- Out-proj: token-parallel after per-head 4-rank AllToAll (two replica groups,
  one per batch - no cross-batch zero padding). Out-proj is split in two
  passes: contraction blocks from heads 0/1 are accumulated mid-attention, the
  rest plus bias in a short tail, summed with the stashed partials on DVE.
"""

import sys

if '/opt/trn_rl_repo' not in sys.path:
    sys.path.insert(0, '/opt/trn_rl_repo')

import numpy as np

P = 128
B, S, E, H, DH = 2, 2048, 1024, 16, 64
NCORES = 8
G = 4                 # head groups == cores per batch
EG = E // G           # 256 emb cols per group
TS = S // G           # 512 tokens per core in out-proj
KB = S // P           # 16 key-token blocks
IB = E // P           # 8 contraction blocks of 128
QW = 512              # matmul moving free-dim chunk (one PSUM bank of f32)
HW = 1024             # exp tile q width
SCALE = DH ** -0.5

_cache = {}


def _build():
    import concourse.mybir as mybir
    import concourse.tile as tile
    from concourse import bacc
    from contextlib import ExitStack

    f32 = mybir.dt.float32
    bf16 = mybir.dt.bfloat16
    AF = mybir.ActivationFunctionType

    nc = bacc.Bacc("TRN2", target_bir_lowering=False, debug=False,
                   num_devices=NCORES)

    xqT = nc.dram_tensor("xqT", [E, S], bf16, kind="ExternalInput").ap()
    xkT = nc.dram_tensor("xkT", [E, S], bf16, kind="ExternalInput").ap()
    xvT = nc.dram_tensor("xvT", [E, S], bf16, kind="ExternalInput").ap()
    wqT = nc.dram_tensor("wqT", [E, EG], bf16, kind="ExternalInput").ap()
    wkT = nc.dram_tensor("wkT", [E, EG], bf16, kind="ExternalInput").ap()
    wvT = nc.dram_tensor("wvT", [E, EG], bf16, kind="ExternalInput").ap()
    woT = nc.dram_tensor("woT", [E, E], bf16, kind="ExternalInput").ap()
    bqv = nc.dram_tensor("bqv", [1, EG], bf16, kind="ExternalInput").ap()
    bkv = nc.dram_tensor("bkv", [1, EG], bf16, kind="ExternalInput").ap()
    bvv = nc.dram_tensor("bvv", [1, EG], bf16, kind="ExternalInput").ap()
    bov = nc.dram_tensor("bov", [1, E], bf16, kind="ExternalInput").ap()
    onesb = nc.dram_tensor("onesb", [1, S], bf16, kind="ExternalInput").ap()
    mask_pb = nc.dram_tensor("mask_pb", [P, KB], f32, kind="ExternalInput").ap()
    maskrep = nc.dram_tensor("maskrep", [P, KB * G], f32, kind="ExternalInput").ap()
    zmask = nc.dram_tensor("zmask", [P, 2], f32, kind="ExternalInput").ap()
    out = nc.dram_tensor("out", [TS, E], f32, kind="ExternalOutput").ap()

    a2a_ins = [nc.dram_tensor(f"a2a_in{h}", [NCORES, DH, TS], bf16).ap()
               for h in range(G)]
    a2a_outs = [nc.dram_tensor(f"a2a_out{h}", [NCORES, DH, TS], bf16).ap()
                for h in range(G)]
    groups = [list(range(NCORES))]

    DEBUG = _cache.get('debug', False)
    if DEBUG:
        dbg = {
            "d_vp0": nc.dram_tensor("d_vp0", [P, G * (DH + 1)], bf16,
                                    kind="ExternalOutput").ap(),
            "d_kpT0": nc.dram_tensor("d_kpT0", [P, S], bf16,
                                     kind="ExternalOutput").ap(),
            "d_qpT0": nc.dram_tensor("d_qpT0", [P, S], bf16,
                                     kind="ExternalOutput").ap(),
            "d_cpv": nc.dram_tensor("d_cpv", [DH + 1, HW], f32,
                                    kind="ExternalOutput").ap(),
            "d_rec": nc.dram_tensor("d_rec", [1, HW], f32,
                                    kind="ExternalOutput").ap(),
            "d_ctx": nc.dram_tensor("d_ctx", [DH + 1, HW], bf16,
                                    kind="ExternalOutput").ap(),
            "d_es0": nc.dram_tensor("d_es0", [P, HW], bf16,
                                    kind="ExternalOutput").ap(),
            "d_ga0": nc.dram_tensor("d_ga0", [P, TS], bf16,
                                    kind="ExternalOutput").ap(),
        }

    with tile.TileContext(nc) as tc, ExitStack() as top:
        const = top.enter_context(tc.tile_pool(name="const", bufs=1))

        ones_b = const.tile([1, S], bf16)
        nc.sync.dma_start(ones_b[:], onesb[:])
        bq_r = const.tile([1, EG], bf16)
        nc.sync.dma_start(bq_r[:], bqv[:])
        bk_r = const.tile([1, EG], bf16)
        nc.sync.dma_start(bk_r[:], bkv[:])
        bv_r = const.tile([1, EG], bf16)
        nc.sync.dma_start(bv_r[:], bvv[:])
        bo_r = const.tile([1, E], bf16)
        nc.sync.dma_start(bo_r[:], bov[:])
        mask_t = const.tile([P, KB], f32)
        nc.sync.dma_start(mask_t[:], mask_pb[:])
        maskrep_t = const.tile([P, KB * G], f32)
        nc.sync.dma_start(maskrep_t[:], maskrep[:])
        zmask_t = const.tile([P, 2], f32)
        nc.sync.dma_start(zmask_t[:], zmask[:])

        # persistent SBUF
        proj_sb = top.enter_context(tc.tile_pool(name="proj_sb", bufs=1))
        qpT = [proj_sb.tile([P, S], bf16, tag=f"qpT{m}", name=f"qpT{m}")
               for m in range(2)]
        kpT = [proj_sb.tile([P, S], bf16, tag=f"kpT{m}", name=f"kpT{m}")
               for m in range(2)]
        vpe = [proj_sb.tile([P, S], bf16, tag=f"vpe{m}", name=f"vpe{m}")
               for m in range(2)]
        vp_sb = [proj_sb.tile([P, G * (DH + 1)], bf16, tag=f"vp{j}", name=f"vp{j}")
                 for j in range(KB)]
        ga = [proj_sb.tile([P, TS], bf16, tag=f"ga{ib}", name=f"ga{ib}")
              for ib in range(IB)]
        partial = [proj_sb.tile([P, E], f32, tag=f"part{tm}", name=f"part{tm}")
                   for tm in range(TS // P)]

        # projection + out-proj weights
        w_pool = top.enter_context(tc.tile_pool(name="wqkv", bufs=1))
        w_r = {}
        for name, wap in (("v", wvT), ("k", wkT), ("q", wqT)):
            wr = w_pool.tile([P, IB * EG], bf16, tag=f"w{name}r", name=f"w{name}r")
            nc.sync.dma_start(wr[:].rearrange("p (i e) -> p i e", e=EG),
                              wap.rearrange("(i p) e -> p i e", p=P))
            w_r[name] = wr
        wo_r = w_pool.tile([P, IB * E], bf16, tag="wor", name="wor")

        # ---- projections: V first (vp feeds PV), then K, then Q ----
        # deep x prefetch: dma dispatches don't park behind tile-reuse waits,
        # so transfers overlap and the PE stays densely fed (high p-state)
        with tc.tile_pool(name="xst", bufs=8) as xst, \
             tc.tile_pool(name="kqpsum", bufs=1, space="PSUM") as kqpsum:
            for name, xap, brow in (("v", xvT, bv_r), ("k", xkT, bk_r),
                                    ("q", xqT, bq_r)):
                kqs = [kqpsum.tile([P, S], f32, tag=f"kqs{m}", name=f"kqs{m}")
                       for m in range(2)]
                for i in range(IB):
                    xr = xst.tile([P, S], bf16)
                    nc.sync.dma_start(xr[:], xap[i * P:(i + 1) * P, :])
                    for m in range(2):
                        for c in range(S // QW):
                            nc.tensor.matmul(
                                kqs[m][:, c * QW:(c + 1) * QW],
                                w_r[name][:, i * EG + m * P:i * EG + (m + 1) * P],
                                xr[:, c * QW:(c + 1) * QW],
                                start=(i == 0), stop=False)
                for m in range(2):
                    for c in range(S // QW):
                        nc.tensor.matmul(
                            kqs[m][:, c * QW:(c + 1) * QW],
                            brow[0:1, m * P:(m + 1) * P],
                            ones_b[0:1, c * QW:(c + 1) * QW],
                            start=False, stop=True)
                dsts = {"v": vpe, "k": kpT, "q": qpT}[name]
                for m in range(2):
                    nc.vector.tensor_copy(dsts[m][:], kqs[m][:])

        # out-proj weights: emitted after x so the queues drain x first
        nc.sync.dma_start(wo_r[:].rearrange("p (i e) -> p i e", e=E),
                          woT.rearrange("(i p) e -> p i e", p=P))

        # token-major vp tiles via DMA xbar transpose (contiguous dst only —
        # strided xbar dst scrambles), then DVE copy into the head-block
        # layout. Emitted AFTER all x loads: the transposes wait on vpe in
        # the in-order SP queue, and anything queued behind them would stall.
        # mask/ones column sits at slot 0 of each head block so sumexp lands
        # in PSUM partition 0 (reciprocal_approx_fast needs partition 0).
        tokp = top.enter_context(tc.tile_pool(name="tokp", bufs=4))
        for j in range(KB):
            for m in range(2):
                tok = tokp.tile([P, P], bf16)
                nc.sync.dma_start_transpose(
                    tok[:], vpe[m][:, j * P:(j + 1) * P])
                dst3 = vp_sb[j].rearrange(
                    "p (h e) -> p h e", e=DH + 1)[:, 2 * m:2 * m + 2, 1:DH + 1]
                nc.vector.tensor_copy(
                    dst3, tok.rearrange("p (h e) -> p h e", e=DH))
            nc.vector.tensor_copy(
                vp_sb[j].rearrange("p (h e) -> p h e", e=DH + 1)[:, :, 0:1],
                maskrep_t[:, j * G:(j + 1) * G].rearrange(
                    "p (h e) -> p h e", e=1))
            vals = vp_sb[j].rearrange(
                "p (h e) -> p h e", e=DH + 1)[:, :, 1:DH + 1]
            nc.vector.tensor_scalar_mul(vals, vals, mask_t[:, j:j + 1])

        # ---- attention: flat pipelined stream over (head, q-half, kblock) ----
        flat = [(h, half, j)
                for h in range(G) for half in range(2) for j in range(KB)]

        with tc.tile_pool(name="spsum", bufs=2, space="PSUM") as spsum, \
             tc.tile_pool(name="pvpsum", bufs=1, space="PSUM") as pvpsum, \
             tc.tile_pool(name="opsum", bufs=1, space="PSUM") as opsum, \
             tc.tile_pool(name="expp", bufs=8) as expp, \
             tc.tile_pool(name="cpvp", bufs=2) as cpvp, \
             tc.tile_pool(name="normp", bufs=2) as normp, \
             tc.tile_pool(name="ctxp", bufs=2) as ctxp, \
             tc.tile_pool(name="foutp", bufs=2) as foutp:

            sp_t = {}
            es_t = {}
            pv_t = {}
            oproj_chunks = []

            def emit_scores(u):
                h, half, j = flat[u]
                mt, po = h // 2, (h % 2) * DH
                sp = spsum.tile([P, HW], f32, tag="sp", name=f"sp{u}")
                for c in range(HW // QW):
                    q0 = half * HW + c * QW
                    nc.tensor.matmul(
                        sp[:, c * QW:(c + 1) * QW],
                        kpT[mt][po:po + DH, j * P:(j + 1) * P],
                        qpT[mt][po:po + DH, q0:q0 + QW],
                        start=True, stop=True)
                sp_t[u] = sp

            def emit_exp(u):
                es = expp.tile([P, HW], bf16)
                nc.scalar.activation(es[:], sp_t.pop(u)[:], AF.Exp, scale=SCALE)
                es_t[u] = es
                if DEBUG and u == 0:
                    nc.sync.dma_start(dbg["d_es0"][:], es[:])

            def emit_pv(u):
                h, half, j = flat[u]
                if j == 0:
                    pv_t[(h, half)] = pvpsum.tile([DH + 1, HW], f32, tag="pv",
                                                  name=f"pv{h}_{half}")
                pv = pv_t[(h, half)]
                es = es_t.pop(u)
                for c in range(HW // QW):
                    nc.tensor.matmul(
                        pv[:, c * QW:(c + 1) * QW],
                        vp_sb[j][:, h * (DH + 1):(h + 1) * (DH + 1)],
                        es[:, c * QW:(c + 1) * QW],
                        start=(j == 0), stop=(j == KB - 1))
                if j == KB - 1:
                    emit_norm_send(h, half)

            def emit_norm_send(h, half):
                # copy pv out of PSUM first so the single pv buffer frees fast
                pv = pv_t.pop((h, half))
                cpv = cpvp.tile([DH + 1, HW], f32)
                nc.vector.tensor_copy(cpv[:], pv[:])
                rec = normp.tile([1, HW], f32, tag="rec", name="rec")
                nc.vector.reciprocal_approx_fast(rec[:], cpv[0:1, :])
                recB = normp.tile([DH + 1, HW], f32, tag="recB", name="recB")
                nc.gpsimd.partition_broadcast(recB[:], rec[:])
                # row 0 becomes sumexp/sumexp ~ 1; rows 1..64 the context
                ctx = ctxp.tile([DH + 1, HW], bf16)
                nc.vector.tensor_mul(ctx[:], cpv[:], recB[:])
                # slot pairs (tsl, tsl+4): exactly one is this batch's rank;
                # the host 0/1 zmask zeroes the other so the 8-rank AllToAll
                # stays SPMD-uniform
                ctxA = ctxp.tile([DH + 1, HW], bf16, tag="ctxA", name="ctxA")
                nc.vector.tensor_scalar_mul(ctxA[:], ctx[:], zmask_t[0:DH + 1, 0:1])
                ctxB = ctxp.tile([DH + 1, HW], bf16, tag="ctxB", name="ctxB")
                nc.vector.tensor_scalar_mul(ctxB[:], ctx[:], zmask_t[0:DH + 1, 1:2])
                if DEBUG and (h, half) == (0, 0):
                    nc.sync.dma_start(dbg["d_cpv"][:], cpv[:])
                    nc.sync.dma_start(dbg["d_rec"][:], rec[:])
                    nc.sync.dma_start(dbg["d_ctx"][:], ctx[:])
                for jj in range(2):
                    tsl = half * 2 + jj
                    nc.sync.dma_start(a2a_ins[h][tsl],
                                      ctxA[1:DH + 1, jj * TS:(jj + 1) * TS])
                    nc.sync.dma_start(a2a_ins[h][tsl + 4],
                                      ctxB[1:DH + 1, jj * TS:(jj + 1) * TS])
                if half == 1:
                    nc.gpsimd.collective_compute(
                        "AllToAll", mybir.AluOpType.bypass,
                        replica_groups=groups,
                        ins=[a2a_ins[h][:]], outs=[a2a_outs[h][:]])
                    m, r0 = h // 2, (h % 2) * DH
                    for gp in range(G):
                        gs0 = ctxp.tile([DH, TS], bf16, tag="gs0", name="gs0")
                        gs1 = ctxp.tile([DH, TS], bf16, tag="gs1", name="gs1")
                        nc.sync.dma_start(gs0[:], a2a_outs[h][gp])
                        nc.sync.dma_start(gs1[:], a2a_outs[h][gp + 4])
                        nc.vector.tensor_add(ga[gp * 2 + m][r0:r0 + DH, :],
                                             gs0[:], gs1[:])

            IBS1 = [0, 2, 4, 6]   # contraction blocks from heads 0,1

            def emit_oproj_chunk(tm, n, oc):
                # pass 1, one matmul at a time, interleaved into the attention
                # stream so the in-order tensor queue never parks behind a
                # collective wait and never starves the exp stream
                ib = IBS1[n]
                if n == 0 and oc == 0:
                    oproj_pot[0] = opsum.tile([P, E], f32, tag="pot",
                                              name=f"pot{tm}")
                pot = oproj_pot[0]
                nc.tensor.matmul(
                    pot[:, oc * QW:(oc + 1) * QW],
                    ga[ib][:, tm * P:(tm + 1) * P],
                    wo_r[:, ib * E + oc * QW:ib * E + oc * QW + QW],
                    start=(n == 0), stop=(n == len(IBS1) - 1))
                if n == len(IBS1) - 1 and oc == E // QW - 1:
                    nc.vector.tensor_copy(partial[tm][:], pot[:])

            oproj_pot = [None]

            def emit_oproj_pass2():
                for tm in range(TS // P):
                    pot = opsum.tile([P, E], f32, tag="pot", name=f"pot2_{tm}")
                    for n, ib in enumerate([1, 3, 5, 7]):
                        for oc in range(E // QW):
                            nc.tensor.matmul(
                                pot[:, oc * QW:(oc + 1) * QW],
                                ga[ib][:, tm * P:(tm + 1) * P],
                                wo_r[:, ib * E + oc * QW:ib * E + oc * QW + QW],
                                start=(n == 0), stop=False)
                    for oc in range(E // QW):
                        nc.tensor.matmul(
                            pot[:, oc * QW:(oc + 1) * QW],
                            ones_b[0:1, 0:P],
                            bo_r[0:1, oc * QW:(oc + 1) * QW],
                            start=False, stop=True)
                    fout = foutp.tile([P, E], f32)
                    nc.vector.tensor_add(fout[:], pot[:], partial[tm][:])
                    nc.sync.dma_start(out[tm * P:(tm + 1) * P, :], fout[:])

            # PV lags the exp stream so tensor work comes in contiguous
            # bursts (scores u+1 then pv u-LAG back to back, no short waits)
            PV_LAG = 4
            # pass-1 chunks start well after head 1's AllToAll completes
            OPROJ_U0 = 92
            oproj_chunks = [(tm, n, oc) for tm in range(TS // P)
                            for n in range(len(IBS1)) for oc in range(E // QW)]

            def emit_filler(u):
                # harmless scratch matmul: keeps the tensor engine's busy
                # density high so the PE clock stays at max p-state (measured:
                # sparse attention-only matmul streams settle at half clock)
                scr = opsum.tile([P, E], f32, tag="pot", name=f"fill{u}")
                nc.tensor.matmul(scr[:, 0:QW], kpT[0][:, 0:P],
                                 qpT[0][:, 0:QW], start=True, stop=True)

            emit_scores(0)
            for u in range(len(flat)):
                if u + 1 < len(flat):
                    emit_scores(u + 1)
                emit_exp(u)
                if u >= PV_LAG:
                    emit_pv(u - PV_LAG)
                if u >= OPROJ_U0 and oproj_chunks:
                    emit_oproj_chunk(*oproj_chunks.pop(0))
                elif u < OPROJ_U0:
                    emit_filler(u)
            for u in range(len(flat) - PV_LAG, len(flat)):
                emit_pv(u)
            while oproj_chunks:
                emit_oproj_chunk(*oproj_chunks.pop(0))
            emit_oproj_pass2()
            if DEBUG:
                nc.sync.dma_start(dbg["d_vp0"][:], vp_sb[0][:])
                nc.sync.dma_start(dbg["d_kpT0"][:], kpT[0][:])
                nc.sync.dma_start(dbg["d_qpT0"][:], qpT[0][:])
                nc.sync.dma_start(dbg["d_ga0"][:], ga[0][:])

    nc.compile()
    return nc


def _get_nc():
    if 'nc' not in _cache:
        _cache['nc'] = _build()
    return _cache['nc']


def kernel(q, k, v, mask, Wq, bq, Wk, bk, Wv, bv, Wo, bo):
    import ml_dtypes
    from concourse.bass_utils import run_bass_kernel_spmd

    nc = _get_nc()
    bft = ml_dtypes.bfloat16
    f32 = np.float32

    def _cvt(x):
        return np.ascontiguousarray(np.asarray(x, f32)).astype(bft)

    q = np.asarray(q, f32)
    k = np.asarray(k, f32)
    v = np.asarray(v, f32)
    qT = [_cvt(q[b].T) for b in range(B)]
    kT = [_cvt(k[b].T) for b in range(B)]
    vT = [_cvt(v[b].T) for b in range(B)]
    WqT = _cvt(np.asarray(Wq, f32).T)
    WkT = _cvt(np.asarray(Wk, f32).T)
    WvT = _cvt(np.asarray(Wv, f32).T)
    WoT = _cvt(np.asarray(Wo, f32).T)
    bq = _cvt(bq)
    bk = _cvt(bk)
    bv = _cvt(bv)
    bo = _cvt(bo)
    onesbv = np.ones((1, S), bft)
    maskf = (np.asarray(mask) != 0).astype(f32)  # [B, S]

    in_maps = []
    for r in range(NCORES):
        b, g = r // G, r % G
        cols = slice(g * EG, (g + 1) * EG)
        m_pb = np.ascontiguousarray(maskf[b].reshape(KB, P).T)       # [128,16]
        m_rep = np.ascontiguousarray(np.repeat(m_pb, G, axis=1))     # [128,64]
        zm = np.zeros((P, 2), f32)
        zm[:, b] = 1.0
        in_maps.append({
            "xqT": qT[b], "xkT": kT[b], "xvT": vT[b],
            "wqT": np.ascontiguousarray(WqT[:, cols]),
            "wkT": np.ascontiguousarray(WkT[:, cols]),
            "wvT": np.ascontiguousarray(WvT[:, cols]),
            "woT": WoT,
            "bqv": bq[None, cols], "bkv": bk[None, cols],
            "bvv": bv[None, cols], "bov": bo[None, :],
            "onesb": onesbv,
            "mask_pb": m_pb, "maskrep": m_rep, "zmask": zm,
        })

    res = run_bass_kernel_spmd(nc, in_maps, core_ids=list(range(NCORES)),
                               **_cache.get('run_kwargs', {}))
    _cache['last_results'] = res

    full = np.empty((B, S, E), f32)
    for r in range(NCORES):
        b, g = r // G, r % G
        full[b, g * TS:(g + 1) * TS, :] = res.results[r]["out"]
    return full
